# revision 3
# baseline (speedup 1.0000x reference)
"""Deformable Conv1d (B=4, C=256, L=8192, K=3, DG=4) on 8 Trainium2 cores.

v3: fully software-pipelined over wave slots. Sharding: core = (sample
b = core//2, L-half h = core%2); each core computes out[b, :, h*4096:].

Structure (vs the staged v1/v2): the two 2048-column waves per rep are
flattened into a stream of wave slots. Each slot's body loop (8 pair
iterations) interleaves, per iteration:
  - 8 gathers for a dk-pair, issued A0(d0) A0(d1) B0(d0) B0(d1) A1(d0)...
    so same-tile siblings sit 4 issue slots apart and their whole-tensor
    WAW (gated on the DMA-completion semaphore, ~2.8us) is hidden.
  - 4 chase steps of the PREVIOUS slot's cp 2,3 main matmuls (sequential
    (cp, mb) pairs, one aux PSUM tile at a time) + its output DMA.
  - modulate (sel broadcast matmuls -> Act drains -> DVE muls + pair-add)
    for the previous pair.
  - one kb accumulation step of this slot's cp 0,1 main matmuls.
  - one piece of the NEXT slot's front (conv / chain / idx spread), so
    wave fronts ride inside the previous slot instead of serializing.
Rep-level tiles (aplane, shalf, idxw, ow) allocate at even slots; the
aplane zero rows are memset once (buffers persist across reps).
"""
import os
# Subtile dependency tracking misses deps for strided/rearranged APs
# (sh_sig/ap_sig); whole-tensor deps stay on by default. idxw tiles opt
# back in: their writers/readers are plain slices and whole-tensor WAW
# would chain the 12 spread DMAs on each other's DMA completions.
os.environ.setdefault("BY_DEFAULT_DISABLE_SUBTILE_DEPS", "1")
import sys
sys.path.insert(0, '/opt/trn_rl_repo')
from contextlib import ExitStack
import numpy as np
import ml_dtypes

import concourse.bass as bass
import concourse.tile as tile
from concourse import bacc, mybir

dt = mybir.dt
bf16 = ml_dtypes.bfloat16

B, C, L = 4, 256, 8192
N_CORES = 8
LH = L // 2
HALO = 17
W = LH + 2 * HALO          # 4130 window positions
WROWS = 33 * 128           # 4224 padded rows in pair tables
WAVE = 2048
AF = mybir.ActivationFunctionType
ALU = mybir.AluOpType


def build_program(n_reps=1):
    nc = bacc.Bacc("TRN2", target_bir_lowering=False, debug=False,
                   enable_asserts=True, num_devices=N_CORES,
                   num_swdge_queues=4, dynamic_dma_scratch_size=24576)

    def din(name, shape, dty):
        return nc.dram_tensor(name, shape, dty, kind="ExternalInput").ap()

    xT = din("xT", (4, 128, WROWS), dt.bfloat16)
    xP = din("xP", (2, 128, W), dt.bfloat16)
    wconv = din("wconv", (2, 3, 128, 64), dt.bfloat16)
    iotas = din("iotas", (2, 128, 512), dt.float32)
    boff = din("boff", (32, 1), dt.float32)
    bmask = din("bmask", (32, 1), dt.float32)
    wmain = din("wmain", (6, 2, 128, 128), dt.bfloat16)
    bmain = din("bmain", (2, 128, 1), dt.float32)
    wsel = din("wsel", (12, 64, 128), dt.bfloat16)
    yout = nc.dram_tensor("y", (2, 128, LH), dt.bfloat16,
                          kind="ExternalOutput").ap()

    with ExitStack() as ctx:
        tc = ctx.enter_context(tile.TileContext(nc))
        cpool = ctx.enter_context(tc.tile_pool(name="const", bufs=1))
        chpool = ctx.enter_context(tc.tile_pool(name="chain", bufs=2))
        splane = ctx.enter_context(tc.tile_pool(name="spl", bufs=1))
        gpool = ctx.enter_context(tc.tile_pool(name="g", bufs=2))
        apool = ctx.enter_context(tc.tile_pool(name="a", bufs=2))
        mpool = ctx.enter_context(tc.tile_pool(name="mtp", bufs=1))
        opool = ctx.enter_context(tc.tile_pool(name="o", bufs=1))
        aux = ctx.enter_context(tc.tile_pool(name="aux", bufs=2, space="PSUM"))
        psb = ctx.enter_context(tc.tile_pool(name="psb", bufs=2, space="PSUM"))
        psm = ctx.enter_context(tc.tile_pool(name="psm", bufs=1, space="PSUM"))

        t_xT = [cpool.tile([128, WROWS], dt.bfloat16, tag=f"xT{d}", name=f"xT{d}")
                for d in range(4)]
        for d in range(4):
            nc.sync.dma_start(t_xT[d][:], xT[d])
        t_xP = [cpool.tile([128, W], dt.bfloat16, tag=f"xP{cb}", name=f"xP{cb}")
                for cb in range(2)]
        for cb in range(2):
            nc.sync.dma_start(t_xP[cb][:], xP[cb])
        t_wconv = [[cpool.tile([128, 64], dt.bfloat16, tag=f"wc{cb}{k}",
                               name=f"wc{cb}{k}") for k in range(3)]
                   for cb in range(2)]
        for cb in range(2):
            for k in range(3):
                nc.sync.dma_start(t_wconv[cb][k][:], wconv[cb, k])
        t_iot = [cpool.tile([128, 512], dt.float32, tag=f"iot{t}", name=f"iot{t}")
                 for t in range(2)]
        for t in range(2):
            nc.sync.dma_start(t_iot[t][:], iotas[t])
        t_boff = cpool.tile([32, 1], dt.float32, name="boff")
        nc.sync.dma_start(t_boff[:], boff[:])
        t_bmask = cpool.tile([32, 1], dt.float32, name="bmask")
        nc.sync.dma_start(t_bmask[:], bmask[:])
        t_wmain = [[cpool.tile([128, 128], dt.bfloat16, tag=f"wm{kb}{mb}",
                               name=f"wm{kb}{mb}") for mb in range(2)]
                   for kb in range(6)]
        for kb in range(6):
            for mb in range(2):
                nc.sync.dma_start(t_wmain[kb][mb][:], wmain[kb, mb])
        t_bmain = [cpool.tile([128, 1], dt.float32, tag=f"bm{mb}", name=f"bm{mb}")
                   for mb in range(2)]
        for mb in range(2):
            nc.sync.dma_start(t_bmain[mb][:], bmain[mb])
        t_wsel = [cpool.tile([64, 128], dt.bfloat16, tag=f"sel{dk}",
                             name=f"sel{dk}") for dk in range(12)]
        for dk in range(12):
            nc.sync.dma_start(t_wsel[dk][:], wsel[dk])
        # shared num_idxs register (a per-gather RegisterMove would WAR
        # against in-flight gathers)
        nreg = nc.gpsimd.to_reg(512)

        slots = 2 * n_reps
        st = {}          # per-slot state
        rep_state = [None]

        def emit_front_pieces(s):
            """Build the 7 front pieces (conv x4, chain x2, spread) for
            slot s. Rep-level tiles allocate on even slots."""
            w = s % 2
            if w == 0:
                rs = {}
                rs['idxw'] = []
                for ww in range(2):
                    t = splane.tile([128, 1536], dt.int16, tag=f"idx{ww}",
                                    name=f"idx{ww}")
                    splane.parent.tiles[-1].subtile_deps = True
                    rs['idxw'].append(t)
                rs['ow'] = [[opool.tile([128, WAVE], dt.bfloat16,
                                        tag=f"ow{ww}{mb}", name=f"ow{ww}{mb}")
                             for mb in range(2)] for ww in range(2)]
                rep_state[0] = rs
            rs = rep_state[0]
            # aplane/shalf are PER-WAVE-PARITY tensors: a whole-rep tensor
            # would WAR-serialize this wave's chain writes against every
            # sel-matmul/chase read of the previous wave (6-13us Pool
            # stalls at slot boundaries).
            apl = splane.tile([64, WAVE], dt.bfloat16, tag=f"apl{w}",
                              name=f"apl{w}")
            if s <= 1:
                # zero rows contract against zero selector weights; buffers
                # persist across reps, so memset each parity once
                nc.gpsimd.memset(apl[:], 0.0)
            shalfw = [splane.tile([128, WAVE], dt.bfloat16,
                                  tag=f"s{kb}_{w}", name=f"s{kb}_{w}")
                      for kb in range(6)]
            st[s] = {'rs': rs, 'w': w, 'aplane': apl,
                     'ap_sig': apl[:].rearrange("a (p u h) -> a u h p",
                                                p=16, u=4, h=32),
                     'shalf': shalfw,
                     'sh_sig': [shalfw[kb][:].rearrange(
                         "a (r g) -> a g r", r=128, g=16)
                         for kb in range(6)]}
            cv = {}

            def conv_piece(cb):
                def go():
                    if cb == 0:
                        cv['pk'] = chpool.tile([128, 512], dt.float32,
                                               tag="pk", name="pk", bufs=1)
                        cv['mk'] = chpool.tile([128, 512], dt.float32,
                                               tag="mk", name="mk", bufs=1)
                    c = 4 * w + cb
                    ps = aux.tile([64, 512], dt.float32, tag="aux",
                                  name="convps")
                    for xb in range(2):
                        for k in range(3):
                            rhs = t_xP[xb][:, c * 512 + HALO - 1 + k:
                                           c * 512 + HALO - 1 + k + 512]
                            nc.tensor.matmul(ps[:], t_wconv[xb][k][:], rhs,
                                             start=(xb == 0 and k == 0),
                                             stop=(xb == 1 and k == 2))
                    rb = 32 * cb
                    nc.scalar.activation(cv['pk'][rb:rb + 32, :], ps[0:32, :],
                                         AF.Identity, bias=t_boff[:],
                                         scale=1.0)
                    nc.scalar.activation(cv['mk'][rb:rb + 32, :], ps[32:64, :],
                                         AF.Identity, bias=t_bmask[:],
                                         scale=1.0)
                    if cb == 3:
                        nc.scalar.activation(cv['mk'][:], cv['mk'][:],
                                             AF.Sigmoid)
                return go

            def chain1():
                pk, mk = cv['pk'], cv['mk']
                cv['i16r'] = chpool.tile([128, 512], dt.int16, tag="i16r",
                                         name="i16r", bufs=1)
                cv['p0f'] = chpool.tile([128, 512], dt.float32, tag="p0f",
                                        name="p0f", bufs=1)
                cv['ttl'] = chpool.tile([128, 512], dt.float32, tag="ttl",
                                        name="ttl", bufs=1)
                cv['msk'] = chpool.tile([128, 512], dt.float32, tag="msk",
                                        name="msk", bufs=1)
                nc.vector.tensor_add(pk[:], pk[:], t_iot[w][:])
                nc.scalar.copy(cv['i16r'][:], pk[:])      # round to nearest
                nc.scalar.copy(cv['p0f'][:], cv['i16r'][:])
                nc.vector.tensor_sub(cv['ttl'][:], pk[:], cv['p0f'][:])
                nc.vector.tensor_scalar(cv['msk'][:], cv['ttl'][:], 0.0,
                                        None, ALU.is_lt)
                nc.vector.tensor_sub(cv['p0f'][:], cv['p0f'][:], cv['msk'][:])
                nc.vector.tensor_add(cv['ttl'][:], cv['ttl'][:], cv['msk'][:])

            def chain2():
                mk, ttl, p0f = cv['mk'], cv['ttl'], cv['p0f']
                aplane = st[s]['aplane']
                cv['i16p'] = chpool.tile([128, 512], dt.int16, tag="i16p",
                                         name="i16p", bufs=1)
                nc.vector.tensor_mul(ttl[:], ttl[:], mk[:])   # t*m
                for cb in range(4):
                    cc = cb * 512
                    nc.scalar.copy(aplane[32:44, cc:cc + 512],
                                   ttl[32 * cb:32 * cb + 12, :])
                    nc.vector.tensor_sub(aplane[0:12, cc:cc + 512],
                                         mk[32 * cb:32 * cb + 12, :],
                                         ttl[32 * cb:32 * cb + 12, :])
                nc.vector.tensor_scalar(cv['i16p'][:], p0f[:], 0.0,
                                        float(W - 1), ALU.max, ALU.min)

            def spread():
                i16p = cv['i16p']
                idxw_w = rs['idxw'][w]
                for dk in range(12):
                    nc.sync.dma_start(idxw_w[0:16, dk * 128:(dk + 1) * 128],
                                      i16p[dk:128:32, :])
                # 7 replicates, all reading [0:16] -> no dependency chain
                for q in range(1, 8):
                    nc.sync.dma_start(idxw_w[16 * q:16 * q + 16, :],
                                      idxw_w[0:16, :])

            def pair01():
                conv_piece(0)()
                conv_piece(1)()

            def pair23():
                conv_piece(2)()
                conv_piece(3)()

            return [pair01, pair23, chain1, chain2, spread]

        def chase_gen(s):
            """Generator of 24 chase steps for slot s's cp 2,3 columns:
            sequential (cp, mb) pairs, one aux tile each."""
            rs, w = st[s]['rs'], st[s]['w']
            for cp in (2, 3):
                for mb in range(2):
                    ct = aux.tile([128, 512], dt.float32, tag="aux",
                                  name="cps")
                    for kb in range(6):
                        nc.tensor.matmul(
                            ct[:], t_wmain[kb][mb][:],
                            st[s]['sh_sig'][kb][:, 4 * cp:4 * cp + 4, :],
                            start=(kb == 0), stop=(kb == 5))
                        if kb == 5:
                            nc.scalar.activation(
                                rs['ow'][w][mb][:, 512 * cp:512 * (cp + 1)],
                                ct[:], AF.Identity, bias=t_bmain[mb][:],
                                scale=1.0)
                        yield

        def ship_out(s):
            rs, w = st[s]['rs'], st[s]['w']
            for mb in range(2):
                nc.sync.dma_start(yout[mb, :, w * WAVE:(w + 1) * WAVE],
                                  rs['ow'][w][mb][:])

        def modulate_pair(s, p):
            rs, w = st[s]['rs'], st[s]['w']
            gth_pair = st[s]['gq'].pop(p)
            for di in range(2):
                dk = 2 * p + di
                kb, h = dk // 2, dk % 2
                gth = gth_pair[di]
                for hf in range(2):
                    ao = 1024 * hf
                    ath = apool.tile([128, 1024], dt.bfloat16,
                                     tag=f"at{hf}", name="ath")
                    for uu in range(2):
                        bps = psb.tile([128, 512], dt.float32, tag="bcps",
                                       name="bcps")
                        nc.tensor.matmul(bps[:], t_wsel[dk][:],
                                         st[s]['ap_sig'][:, 2 * hf + uu],
                                         start=True, stop=True)
                        nc.scalar.copy(ath[:, 512 * uu:512 * (uu + 1)],
                                       bps[:])
                    mt0 = mpool.tile([64, 1024], dt.bfloat16,
                                     tag=f"mt0{hf}", name="mt0")
                    mt1 = mpool.tile([64, 1024], dt.bfloat16,
                                     tag=f"mt1{hf}", name="mt1")
                    nc.vector.tensor_mul(mt0[:], gth[hf][0:64, :],
                                         ath[0:64, :])
                    nc.vector.tensor_mul(mt1[:], gth[hf][64:128, :],
                                         ath[64:128, :])
                    nc.vector.tensor_add(
                        st[s]['shalf'][kb][64 * h:64 * h + 64,
                                           ao:ao + 1024],
                        mt0[:], mt1[:])

        def body(s, fronts_next):
            rs, w = st[s]['rs'], st[s]['w']
            idxw_w = rs['idxw'][w]
            st[s]['gq'] = {}
            mtiles = {(cp, mb): psm.tile([128, 512], dt.float32,
                                         tag=f"mps{cp}{mb}", name="mps")
                      for cp in (0, 1) for mb in range(2)}
            chase = chase_gen(s - 1) if s >= 1 else None
            for pi in range(8):
                if pi < 6:
                    # pair p = pi: dk d0=2p (tags gt0*), d1=2p+1 (gt1*)
                    pair_tiles = []
                    for di in range(2):
                        pair_tiles.append([
                            gpool.tile([128, 1024], dt.bfloat16,
                                       tag=f"gt{di}{hf}", name=f"gt{di}{hf}")
                            for hf in range(2)])
                    st[s]['gq'][pi] = pair_tiles
                    for sib in range(2):
                        for hf in range(2):
                            for di in range(2):
                                dk = 2 * pi + di
                                u = 2 * hf + sib
                                nc.gpsimd.dma_gather(
                                    pair_tiles[di][hf]
                                    [:, 512 * sib:512 * (sib + 1)]
                                    .unsqueeze(1),
                                    t_xT[dk // 3][:],
                                    idxw_w[:, dk * 128 + 32 * u:
                                           dk * 128 + 32 * u + 32],
                                    num_idxs=512, num_idxs_reg=nreg,
                                    elem_size=128, transpose=True,
                                    queue_num=u,
                                    sbuf_tokens_per_rank=128,
                                    sbuf_free_dim_per_rank=256)
                if chase is not None and pi < 6:
                    for _ in range(4):
                        next(chase)
                    if pi == 5:
                        ship_out(s - 1)
                if 1 <= pi <= 6:
                    modulate_pair(s, pi - 1)
                if pi >= 2:
                    kb = pi - 2
                    for cp in (0, 1):
                        for mb in range(2):
                            mps = mtiles[(cp, mb)]
                            nc.tensor.matmul(
                                mps[:], t_wmain[kb][mb][:],
                                st[s]['sh_sig'][kb][:, 4 * cp:4 * cp + 4, :],
                                start=(kb == 0), stop=(kb == 5))
                            if kb == 5:
                                nc.scalar.activation(
                                    rs['ow'][w][mb][:,
                                                    512 * cp:512 * (cp + 1)],
                                    mps[:], AF.Identity,
                                    bias=t_bmain[mb][:], scale=1.0)
                if pi < len(fronts_next):
                    fronts_next[pi]()

        # prologue: front of slot 0 emitted standalone
        fronts = emit_front_pieces(0)
        for piece in fronts:
            piece()
        for s in range(slots):
            fronts_next = emit_front_pieces(s + 1) if s + 1 < slots else []
            body(s, fronts_next)
        # epilogue: chase + ship the last slot
        for _ in chase_gen(slots - 1):
            pass
        ship_out(slots - 1)

    nc.compile()
    return nc


# ---------------------------------------------------------------------------

def _prep_core_inputs(x, w_off, b_off, w_mask, b_mask, weight, bias, b, h):
    q0 = h * LH - HALO
    xpad = np.zeros((C, W + 1), np.float32)
    lo, hi = max(0, q0), min(L, q0 + W + 1)
    xpad[:, lo - q0:hi - q0] = x[b][:, lo:hi]
    xpad_bf = xpad.astype(bf16)

    xT = np.zeros((4, 128, WROWS), bf16)
    for d in range(4):
        rows = np.concatenate([xpad_bf[d * 64:(d + 1) * 64, :W],
                               xpad_bf[d * 64:(d + 1) * 64, 1:W + 1]],
                              axis=0).T           # [W, 128] row e
        full = np.zeros((WROWS, 128), bf16)
        full[:W] = rows
        xT[d] = full.reshape(WROWS // 128, 128, 128).transpose(1, 0, 2) \
                    .reshape(128, WROWS)
    xP = np.ascontiguousarray(xpad_bf[:, :W].reshape(2, 128, W))

    wconv = np.zeros((2, 3, 128, 64), bf16)
    for cb in range(2):
        for k in range(3):
            wconv[cb, k, :, 0:12] = w_off[:, cb * 128:(cb + 1) * 128, k].T
            wconv[cb, k, :, 32:44] = w_mask[:, cb * 128:(cb + 1) * 128, k].T
    iotas = np.zeros((2, 128, 512), np.float32)
    col = np.arange(512, dtype=np.float32)
    for t in range(2):
        for cb in range(4):
            for r in range(12):
                iotas[t, 32 * cb + r, :] = \
                    512 * (4 * t + cb) + col + (r % 3) - 1 + HALO
    boff_c = np.zeros((32, 1), np.float32)
    boff_c[0:12, 0] = b_off.astype(np.float32)
    bmask_c = np.zeros((32, 1), np.float32)
    bmask_c[0:12, 0] = b_mask.astype(np.float32)

    wmain = np.zeros((6, 2, 128, 128), bf16)
    for kb in range(6):
        for half in range(2):
            dk = 2 * kb + half
            d, k = dk // 3, dk % 3
            wblock = weight[:, d * 64:(d + 1) * 64, k]
            for mb in range(2):
                wmain[kb, mb, 64 * half:64 * half + 64, :] = \
                    wblock[mb * 128:(mb + 1) * 128, :].T
    bmain = bias.astype(np.float32).reshape(2, 128, 1)

    wsel = np.zeros((12, 64, 128), bf16)
    for dk in range(12):
        wsel[dk, dk, 0:64] = 1.0
        wsel[dk, 32 + dk, 64:128] = 1.0
    return {"xT": xT, "xP": xP, "wconv": wconv, "iotas": iotas,
            "boff": boff_c, "bmask": bmask_c,
            "wmain": wmain, "bmain": bmain, "wsel": wsel}


_CACHED = {}


def kernel(x, w_off, b_off, w_mask, b_mask, weight, bias):
    x = np.asarray(x, np.float32)
    w_off = np.asarray(w_off, np.float32)
    b_off = np.asarray(b_off, np.float32)
    w_mask = np.asarray(w_mask, np.float32)
    b_mask = np.asarray(b_mask, np.float32)
    weight = np.asarray(weight, np.float32)
    bias = np.asarray(bias, np.float32)

    if "nc" not in _CACHED:
        _CACHED["nc"] = build_program(1)
    nc = _CACHED["nc"]

    in_maps = [
        _prep_core_inputs(x, w_off, b_off, w_mask, b_mask, weight, bias,
                          core // 2, core % 2)
        for core in range(N_CORES)
    ]
    from concourse.bass_utils import run_bass_kernel_spmd
    res = run_bass_kernel_spmd(nc, in_maps, core_ids=list(range(N_CORES)))
    out = np.zeros((B, C, L), np.float32)
    for core in range(N_CORES):
        b, h = core // 2, core % 2
        y = res.results[core]["y"].astype(np.float32)
        out[b, 0:128, h * LH:(h + 1) * LH] = y[0]
        out[b, 128:256, h * LH:(h + 1) * LH] = y[1]
    return out


# revision 4
# speedup vs baseline: 1.0166x; 1.0166x over previous
"""Deformable Conv1d (B=4, C=256, L=8192, K=3, DG=4) on 8 Trainium2 cores.

v3: fully software-pipelined over wave slots. Sharding: core = (sample
b = core//2, L-half h = core%2); each core computes out[b, :, h*4096:].

Structure (vs the staged v1/v2): the two 2048-column waves per rep are
flattened into a stream of wave slots. Each slot's body loop (8 pair
iterations) interleaves, per iteration:
  - 8 gathers for a dk-pair, issued A0(d0) A0(d1) B0(d0) B0(d1) A1(d0)...
    so same-tile siblings sit 4 issue slots apart and their whole-tensor
    WAW (gated on the DMA-completion semaphore, ~2.8us) is hidden.
  - 4 chase steps of the PREVIOUS slot's cp 2,3 main matmuls (sequential
    (cp, mb) pairs, one aux PSUM tile at a time) + its output DMA.
  - modulate (sel broadcast matmuls -> Act drains -> DVE muls + pair-add)
    for the previous pair.
  - one kb accumulation step of this slot's cp 0,1 main matmuls.
  - one piece of the NEXT slot's front (conv / chain / idx spread), so
    wave fronts ride inside the previous slot instead of serializing.
Rep-level tiles (aplane, shalf, idxw, ow) allocate at even slots; the
aplane zero rows are memset once (buffers persist across reps).
"""
import os
# Subtile dependency tracking misses deps for strided/rearranged APs
# (sh_sig/ap_sig); whole-tensor deps stay on by default. idxw tiles opt
# back in: their writers/readers are plain slices and whole-tensor WAW
# would chain the 12 spread DMAs on each other's DMA completions.
os.environ.setdefault("BY_DEFAULT_DISABLE_SUBTILE_DEPS", "1")
import sys
sys.path.insert(0, '/opt/trn_rl_repo')
from contextlib import ExitStack
import numpy as np
import ml_dtypes

import concourse.bass as bass
import concourse.tile as tile
from concourse import bacc, mybir

dt = mybir.dt
bf16 = ml_dtypes.bfloat16

B, C, L = 4, 256, 8192
N_CORES = 8
LH = L // 2
HALO = 17
W = LH + 2 * HALO          # 4130 window positions
WROWS = 33 * 128           # 4224 padded rows in pair tables
WAVE = 2048
AF = mybir.ActivationFunctionType
ALU = mybir.AluOpType


def build_program(n_reps=1):
    nc = bacc.Bacc("TRN2", target_bir_lowering=False, debug=False,
                   enable_asserts=True, num_devices=N_CORES,
                   num_swdge_queues=4, dynamic_dma_scratch_size=24576)

    def din(name, shape, dty):
        return nc.dram_tensor(name, shape, dty, kind="ExternalInput").ap()

    xT = din("xT", (4, 128, WROWS), dt.bfloat16)
    xP = din("xP", (2, 128, W), dt.bfloat16)
    wconv = din("wconv", (2, 3, 128, 64), dt.bfloat16)
    iotas = din("iotas", (2, 128, 512), dt.float32)
    boff = din("boff", (32, 1), dt.float32)
    bmask = din("bmask", (32, 1), dt.float32)
    wmain = din("wmain", (6, 2, 128, 128), dt.bfloat16)
    bmain = din("bmain", (2, 128, 1), dt.float32)
    wsel = din("wsel", (12, 64, 128), dt.bfloat16)
    yout = nc.dram_tensor("y", (2, 128, LH), dt.bfloat16,
                          kind="ExternalOutput").ap()

    with ExitStack() as ctx:
        tc = ctx.enter_context(tile.TileContext(nc))
        cpool = ctx.enter_context(tc.tile_pool(name="const", bufs=1))
        chpool = ctx.enter_context(tc.tile_pool(name="chain", bufs=2))
        splane = ctx.enter_context(tc.tile_pool(name="spl", bufs=1))
        gpool = ctx.enter_context(tc.tile_pool(name="g", bufs=3))
        apool = ctx.enter_context(tc.tile_pool(name="a", bufs=2))
        mpool = ctx.enter_context(tc.tile_pool(name="mtp", bufs=1))
        opool = ctx.enter_context(tc.tile_pool(name="o", bufs=1))
        aux = ctx.enter_context(tc.tile_pool(name="aux", bufs=2, space="PSUM"))
        psb = ctx.enter_context(tc.tile_pool(name="psb", bufs=2, space="PSUM"))
        psm = ctx.enter_context(tc.tile_pool(name="psm", bufs=1, space="PSUM"))

        t_xT = [cpool.tile([128, WROWS], dt.bfloat16, tag=f"xT{d}", name=f"xT{d}")
                for d in range(4)]
        for d in range(4):
            nc.sync.dma_start(t_xT[d][:], xT[d])
        t_xP = [cpool.tile([128, W], dt.bfloat16, tag=f"xP{cb}", name=f"xP{cb}")
                for cb in range(2)]
        for cb in range(2):
            nc.sync.dma_start(t_xP[cb][:], xP[cb])
        t_wconv = [[cpool.tile([128, 64], dt.bfloat16, tag=f"wc{cb}{k}",
                               name=f"wc{cb}{k}") for k in range(3)]
                   for cb in range(2)]
        for cb in range(2):
            for k in range(3):
                nc.sync.dma_start(t_wconv[cb][k][:], wconv[cb, k])
        t_iot = [cpool.tile([128, 512], dt.float32, tag=f"iot{t}", name=f"iot{t}")
                 for t in range(2)]
        for t in range(2):
            nc.sync.dma_start(t_iot[t][:], iotas[t])
        t_boff = cpool.tile([32, 1], dt.float32, name="boff")
        nc.sync.dma_start(t_boff[:], boff[:])
        t_bmask = cpool.tile([32, 1], dt.float32, name="bmask")
        nc.sync.dma_start(t_bmask[:], bmask[:])
        t_wmain = [[cpool.tile([128, 128], dt.bfloat16, tag=f"wm{kb}{mb}",
                               name=f"wm{kb}{mb}") for mb in range(2)]
                   for kb in range(6)]
        for kb in range(6):
            for mb in range(2):
                nc.sync.dma_start(t_wmain[kb][mb][:], wmain[kb, mb])
        t_bmain = [cpool.tile([128, 1], dt.float32, tag=f"bm{mb}", name=f"bm{mb}")
                   for mb in range(2)]
        for mb in range(2):
            nc.sync.dma_start(t_bmain[mb][:], bmain[mb])
        t_wsel = [cpool.tile([64, 128], dt.bfloat16, tag=f"sel{dk}",
                             name=f"sel{dk}") for dk in range(12)]
        for dk in range(12):
            nc.sync.dma_start(t_wsel[dk][:], wsel[dk])
        # shared num_idxs register (a per-gather RegisterMove would WAR
        # against in-flight gathers)
        nreg = nc.gpsimd.to_reg(512)

        slots = 2 * n_reps
        st = {}          # per-slot state
        rep_state = [None]

        def emit_front_pieces(s):
            """Build the 7 front pieces (conv x4, chain x2, spread) for
            slot s. Rep-level tiles allocate on even slots."""
            w = s % 2
            if w == 0:
                rs = {}
                rs['idxw'] = []
                for ww in range(2):
                    t = splane.tile([128, 1536], dt.int16, tag=f"idx{ww}",
                                    name=f"idx{ww}")
                    splane.parent.tiles[-1].subtile_deps = True
                    rs['idxw'].append(t)
                rep_state[0] = rs
            rs = rep_state[0]
            # aplane/shalf are PER-WAVE-PARITY tensors: a whole-rep tensor
            # would WAR-serialize this wave's chain writes against every
            # sel-matmul/chase read of the previous wave (6-13us Pool
            # stalls at slot boundaries).
            apl = splane.tile([64, WAVE], dt.bfloat16, tag=f"apl{w}",
                              name=f"apl{w}")
            if s <= 1:
                # zero rows contract against zero selector weights; buffers
                # persist across reps, so memset each parity once
                nc.gpsimd.memset(apl[:], 0.0)
            shalfw = [splane.tile([128, WAVE], dt.bfloat16,
                                  tag=f"s{kb}_{w}", name=f"s{kb}_{w}")
                      for kb in range(6)]
            # per-slot output tiles, 2 tags x bufs=1: ship_out(s-1) at pi=5
            # always precedes slot s's first drain at pi=7, so one buffer
            # per mb suffices (frees 8KB for the deeper gather rotation)
            oww = [opool.tile([128, WAVE], dt.bfloat16, tag=f"ow{mb}",
                              name=f"ow{mb}") for mb in range(2)]
            st[s] = {'rs': rs, 'w': w, 'aplane': apl, 'ow': oww,
                     'ap_sig': apl[:].rearrange("a (p u h) -> a u h p",
                                                p=16, u=4, h=32),
                     'shalf': shalfw,
                     'sh_sig': [shalfw[kb][:].rearrange(
                         "a (r g) -> a g r", r=128, g=16)
                         for kb in range(6)]}
            cv = {}

            def conv_piece(cb):
                def go():
                    if cb == 0:
                        cv['pk'] = chpool.tile([128, 512], dt.float32,
                                               tag="pk", name="pk", bufs=1)
                        cv['mk'] = chpool.tile([128, 512], dt.float32,
                                               tag="mk", name="mk", bufs=1)
                    c = 4 * w + cb
                    ps = aux.tile([64, 512], dt.float32, tag="aux",
                                  name="convps")
                    for xb in range(2):
                        for k in range(3):
                            rhs = t_xP[xb][:, c * 512 + HALO - 1 + k:
                                           c * 512 + HALO - 1 + k + 512]
                            nc.tensor.matmul(ps[:], t_wconv[xb][k][:], rhs,
                                             start=(xb == 0 and k == 0),
                                             stop=(xb == 1 and k == 2))
                    rb = 32 * cb
                    nc.scalar.activation(cv['pk'][rb:rb + 32, :], ps[0:32, :],
                                         AF.Identity, bias=t_boff[:],
                                         scale=1.0)
                    nc.scalar.activation(cv['mk'][rb:rb + 32, :], ps[32:64, :],
                                         AF.Identity, bias=t_bmask[:],
                                         scale=1.0)
                    if cb == 3:
                        nc.scalar.activation(cv['mk'][:], cv['mk'][:],
                                             AF.Sigmoid)
                return go

            def chain1():
                pk, mk = cv['pk'], cv['mk']
                cv['i16r'] = chpool.tile([128, 512], dt.int16, tag="i16r",
                                         name="i16r", bufs=1)
                cv['p0f'] = chpool.tile([128, 512], dt.float32, tag="p0f",
                                        name="p0f", bufs=1)
                cv['ttl'] = chpool.tile([128, 512], dt.float32, tag="ttl",
                                        name="ttl", bufs=1)
                cv['msk'] = chpool.tile([128, 512], dt.float32, tag="msk",
                                        name="msk", bufs=1)
                nc.vector.tensor_add(pk[:], pk[:], t_iot[w][:])
                nc.scalar.copy(cv['i16r'][:], pk[:])      # round to nearest
                nc.scalar.copy(cv['p0f'][:], cv['i16r'][:])
                nc.vector.tensor_sub(cv['ttl'][:], pk[:], cv['p0f'][:])
                nc.vector.tensor_scalar(cv['msk'][:], cv['ttl'][:], 0.0,
                                        None, ALU.is_lt)
                nc.vector.tensor_sub(cv['p0f'][:], cv['p0f'][:], cv['msk'][:])
                nc.vector.tensor_add(cv['ttl'][:], cv['ttl'][:], cv['msk'][:])

            def chain2():
                mk, ttl, p0f = cv['mk'], cv['ttl'], cv['p0f']
                aplane = st[s]['aplane']
                cv['i16p'] = chpool.tile([128, 512], dt.int16, tag="i16p",
                                         name="i16p", bufs=1)
                nc.vector.tensor_mul(ttl[:], ttl[:], mk[:])   # t*m
                for cb in range(4):
                    cc = cb * 512
                    nc.scalar.copy(aplane[32:44, cc:cc + 512],
                                   ttl[32 * cb:32 * cb + 12, :])
                    nc.vector.tensor_sub(aplane[0:12, cc:cc + 512],
                                         mk[32 * cb:32 * cb + 12, :],
                                         ttl[32 * cb:32 * cb + 12, :])
                nc.vector.tensor_scalar(cv['i16p'][:], p0f[:], 0.0,
                                        float(W - 1), ALU.max, ALU.min)

            def spread():
                i16p = cv['i16p']
                idxw_w = rs['idxw'][w]
                for dk in range(12):
                    nc.sync.dma_start(idxw_w[0:16, dk * 128:(dk + 1) * 128],
                                      i16p[dk:128:32, :])
                # 7 replicates, all reading [0:16] -> no dependency chain
                for q in range(1, 8):
                    nc.sync.dma_start(idxw_w[16 * q:16 * q + 16, :],
                                      idxw_w[0:16, :])

            def pair01():
                conv_piece(0)()
                conv_piece(1)()

            def pair23():
                conv_piece(2)()
                conv_piece(3)()

            return [pair01, pair23, chain1, chain2, spread]

        def chase_gen(s):
            """Generator of 24 chase steps for slot s's cp 2,3 columns:
            sequential (cp, mb) pairs, one aux tile each."""
            rs, w = st[s]['rs'], st[s]['w']
            for cp in (2, 3):
                for mb in range(2):
                    ct = aux.tile([128, 512], dt.float32, tag="aux",
                                  name="cps")
                    for kb in range(6):
                        nc.tensor.matmul(
                            ct[:], t_wmain[kb][mb][:],
                            st[s]['sh_sig'][kb][:, 4 * cp:4 * cp + 4, :],
                            start=(kb == 0), stop=(kb == 5))
                        if kb == 5:
                            nc.scalar.activation(
                                st[s]['ow'][mb][:, 512 * cp:512 * (cp + 1)],
                                ct[:], AF.Identity, bias=t_bmain[mb][:],
                                scale=1.0)
                        yield

        def ship_out(s):
            w = st[s]['w']
            for mb in range(2):
                nc.sync.dma_start(yout[mb, :, w * WAVE:(w + 1) * WAVE],
                                  st[s]['ow'][mb][:])

        def modulate_pair(s, p):
            rs, w = st[s]['rs'], st[s]['w']
            gth_pair = st[s]['gq'].pop(p)
            for di in range(2):
                dk = 2 * p + di
                kb, h = dk // 2, dk % 2
                gth = gth_pair[di]
                for hf in range(2):
                    ao = 1024 * hf
                    ath = apool.tile([128, 1024], dt.bfloat16,
                                     tag=f"at{hf}", name="ath")
                    for uu in range(2):
                        bps = psb.tile([128, 512], dt.float32, tag="bcps",
                                       name="bcps")
                        nc.tensor.matmul(bps[:], t_wsel[dk][:],
                                         st[s]['ap_sig'][:, 2 * hf + uu],
                                         start=True, stop=True)
                        nc.scalar.copy(ath[:, 512 * uu:512 * (uu + 1)],
                                       bps[:])
                    mt0 = mpool.tile([64, 1024], dt.bfloat16,
                                     tag=f"mt0{hf}", name="mt0")
                    mt1 = mpool.tile([64, 1024], dt.bfloat16,
                                     tag=f"mt1{hf}", name="mt1")
                    nc.vector.tensor_mul(mt0[:], gth[hf][0:64, :],
                                         ath[0:64, :])
                    nc.vector.tensor_mul(mt1[:], gth[hf][64:128, :],
                                         ath[64:128, :])
                    nc.vector.tensor_add(
                        st[s]['shalf'][kb][64 * h:64 * h + 64,
                                           ao:ao + 1024],
                        mt0[:], mt1[:])

        def body(s, fronts_next):
            rs, w = st[s]['rs'], st[s]['w']
            idxw_w = rs['idxw'][w]
            st[s]['gq'] = {}
            mtiles = {(cp, mb): psm.tile([128, 512], dt.float32,
                                         tag=f"mps{cp}{mb}", name="mps")
                      for cp in (0, 1) for mb in range(2)}
            chase = chase_gen(s - 1) if s >= 1 else None
            for pi in range(8):
                if pi < 6:
                    # pair p = pi: dk d0=2p (tags gt0*), d1=2p+1 (gt1*)
                    pair_tiles = []
                    for di in range(2):
                        pair_tiles.append([
                            gpool.tile([128, 1024], dt.bfloat16,
                                       tag=f"gt{di}{hf}", name=f"gt{di}{hf}")
                            for hf in range(2)])
                    st[s]['gq'][pi] = pair_tiles
                    for sib in range(2):
                        for hf in range(2):
                            for di in range(2):
                                dk = 2 * pi + di
                                u = 2 * hf + sib
                                nc.gpsimd.dma_gather(
                                    pair_tiles[di][hf]
                                    [:, 512 * sib:512 * (sib + 1)]
                                    .unsqueeze(1),
                                    t_xT[dk // 3][:],
                                    idxw_w[:, dk * 128 + 32 * u:
                                           dk * 128 + 32 * u + 32],
                                    num_idxs=512, num_idxs_reg=nreg,
                                    elem_size=128, transpose=True,
                                    queue_num=u,
                                    sbuf_tokens_per_rank=128,
                                    sbuf_free_dim_per_rank=256)
                if chase is not None and pi < 6:
                    for _ in range(4):
                        next(chase)
                    if pi == 5:
                        ship_out(s - 1)
                if 1 <= pi <= 6:
                    modulate_pair(s, pi - 1)
                if pi >= 2:
                    kb = pi - 2
                    for cp in (0, 1):
                        for mb in range(2):
                            mps = mtiles[(cp, mb)]
                            nc.tensor.matmul(
                                mps[:], t_wmain[kb][mb][:],
                                st[s]['sh_sig'][kb][:, 4 * cp:4 * cp + 4, :],
                                start=(kb == 0), stop=(kb == 5))
                            if kb == 5:
                                nc.scalar.activation(
                                    st[s]['ow'][mb][:,
                                                    512 * cp:512 * (cp + 1)],
                                    mps[:], AF.Identity,
                                    bias=t_bmain[mb][:], scale=1.0)
                if pi < len(fronts_next):
                    fronts_next[pi]()

        # prologue: front of slot 0 emitted standalone
        fronts = emit_front_pieces(0)
        for piece in fronts:
            piece()
        for s in range(slots):
            fronts_next = emit_front_pieces(s + 1) if s + 1 < slots else []
            body(s, fronts_next)
        # epilogue: chase + ship the last slot
        for _ in chase_gen(slots - 1):
            pass
        ship_out(slots - 1)

    nc.compile()
    return nc


# ---------------------------------------------------------------------------

def _prep_core_inputs(x, w_off, b_off, w_mask, b_mask, weight, bias, b, h):
    q0 = h * LH - HALO
    xpad = np.zeros((C, W + 1), np.float32)
    lo, hi = max(0, q0), min(L, q0 + W + 1)
    xpad[:, lo - q0:hi - q0] = x[b][:, lo:hi]
    xpad_bf = xpad.astype(bf16)

    xT = np.zeros((4, 128, WROWS), bf16)
    for d in range(4):
        rows = np.concatenate([xpad_bf[d * 64:(d + 1) * 64, :W],
                               xpad_bf[d * 64:(d + 1) * 64, 1:W + 1]],
                              axis=0).T           # [W, 128] row e
        full = np.zeros((WROWS, 128), bf16)
        full[:W] = rows
        xT[d] = full.reshape(WROWS // 128, 128, 128).transpose(1, 0, 2) \
                    .reshape(128, WROWS)
    xP = np.ascontiguousarray(xpad_bf[:, :W].reshape(2, 128, W))

    wconv = np.zeros((2, 3, 128, 64), bf16)
    for cb in range(2):
        for k in range(3):
            wconv[cb, k, :, 0:12] = w_off[:, cb * 128:(cb + 1) * 128, k].T
            wconv[cb, k, :, 32:44] = w_mask[:, cb * 128:(cb + 1) * 128, k].T
    iotas = np.zeros((2, 128, 512), np.float32)
    col = np.arange(512, dtype=np.float32)
    for t in range(2):
        for cb in range(4):
            for r in range(12):
                iotas[t, 32 * cb + r, :] = \
                    512 * (4 * t + cb) + col + (r % 3) - 1 + HALO
    boff_c = np.zeros((32, 1), np.float32)
    boff_c[0:12, 0] = b_off.astype(np.float32)
    bmask_c = np.zeros((32, 1), np.float32)
    bmask_c[0:12, 0] = b_mask.astype(np.float32)

    wmain = np.zeros((6, 2, 128, 128), bf16)
    for kb in range(6):
        for half in range(2):
            dk = 2 * kb + half
            d, k = dk // 3, dk % 3
            wblock = weight[:, d * 64:(d + 1) * 64, k]
            for mb in range(2):
                wmain[kb, mb, 64 * half:64 * half + 64, :] = \
                    wblock[mb * 128:(mb + 1) * 128, :].T
    bmain = bias.astype(np.float32).reshape(2, 128, 1)

    wsel = np.zeros((12, 64, 128), bf16)
    for dk in range(12):
        wsel[dk, dk, 0:64] = 1.0
        wsel[dk, 32 + dk, 64:128] = 1.0
    return {"xT": xT, "xP": xP, "wconv": wconv, "iotas": iotas,
            "boff": boff_c, "bmask": bmask_c,
            "wmain": wmain, "bmain": bmain, "wsel": wsel}


_CACHED = {}


def kernel(x, w_off, b_off, w_mask, b_mask, weight, bias):
    x = np.asarray(x, np.float32)
    w_off = np.asarray(w_off, np.float32)
    b_off = np.asarray(b_off, np.float32)
    w_mask = np.asarray(w_mask, np.float32)
    b_mask = np.asarray(b_mask, np.float32)
    weight = np.asarray(weight, np.float32)
    bias = np.asarray(bias, np.float32)

    if "nc" not in _CACHED:
        _CACHED["nc"] = build_program(1)
    nc = _CACHED["nc"]

    in_maps = [
        _prep_core_inputs(x, w_off, b_off, w_mask, b_mask, weight, bias,
                          core // 2, core % 2)
        for core in range(N_CORES)
    ]
    from concourse.bass_utils import run_bass_kernel_spmd
    res = run_bass_kernel_spmd(nc, in_maps, core_ids=list(range(N_CORES)))
    out = np.zeros((B, C, L), np.float32)
    for core in range(N_CORES):
        b, h = core // 2, core % 2
        y = res.results[core]["y"].astype(np.float32)
        out[b, 0:128, h * LH:(h + 1) * LH] = y[0]
        out[b, 128:256, h * LH:(h + 1) * LH] = y[1]
    return out
